# revision 33
# baseline (speedup 1.0000x reference)
"""BiDAF attention on Trainium2 — data-parallel over batch across 8 NeuronCores.

Reference math (per batch b):
    sim[c,q] = cq[c] + qq[q] + mm[c,q]
      where cq = ctx @ w_c, qq = qn @ w_q, mm = (ctx * w_m) @ qn^T
    a    = softmax_q(qmask ? sim : -inf)           # [C, Q]
    c2q  = a @ qn                                  # [C, D]
    smax = max_q(sim);  b = softmax_c(cmask ? smax : -inf)
    q2c  = b @ ctx  (broadcast over c)             # [C, D]
    g    = [ctx | c2q | ctx*c2q | ctx*q2c]         # [C, 4D]

Key optimizations vs the f32 baseline (which was HBM-bound at ~60us/core):
  - bf16 end-to-end: inputs are cast host-side, all matmuls/elementwise run
    bf16 (PSUM accumulates f32), the output block is written bf16 and
    upcast host-side. Halves HBM traffic; rel-err stays ~1e-3 << 2e-2.
  - The g1 = ctx block is assembled host-side during unshard (it is a
    verbatim copy of an input): the device writes only [c2q | ctx*c2q |
    ctx*q2c] (3D = 768 per row). Output traffic 16.8 -> 6.3 MB/core.
  - softmax in exp space: ONE exp over simT [65, 512] (bias = qq per
    partition); row 64 of the M1 matmul carries cq (extra lhsT column),
    so after the PE re-transpose, col 64 is exp(cq) and
    e_col = max_q(exp) * exp(cq) * cmask01 needs only tiny DVE ops.
    Masks are 0/1 multiplies after exp (exactly equivalent).
  - q2c uses a replicated-column lhsT (e_col broadcast to [128,128]) so the
    PSUM result is already broadcast over partitions; no separate
    ones-matmul broadcast.
  - Host packs per-row padding: col 256 = 1.0 (sum-via-matmul column),
    col 257 = mask01. Host pre-permutes ctx/g layouts so every DMA line is
    contiguous (2080B / 6144B per partition), 1 DMA per batch per tensor.
  - Elementwise work balanced across ACT/DVE/Pool (~2.6us each per batch).
"""

import numpy as np

import concourse.bass as bass
import concourse.bacc as bacc
import concourse.tile as tile
from concourse import mybir
from concourse.masks import make_identity
from concourse.bass_utils import run_bass_kernel_spmd

B, C, Q, D = 64, 512, 64, 256
N_CORES = 8
BL = B // N_CORES  # batches per core

F32 = mybir.dt.float32
BF16 = mybir.dt.bfloat16
NPBF16 = mybir.dt.np(mybir.dt.bfloat16)
AX = mybir.AxisListType.X
EXP = mybir.ActivationFunctionType.Exp
COPY = mybir.ActivationFunctionType.Copy

NCC = C // 128  # context row chunks (4)
NDC = D // 128  # hidden-dim chunks (2)
DP = D + 4      # padded row: [data(256) | ones | mask01 | 0 | 0]
M1R = Q + 1     # M1 output rows: 64 q rows + 1 cq row


def _emit(tc, ctx_d, qn_d, w_d, g_d, reps=1):
    nc = tc.nc
    with (
        tc.tile_pool(name="consts", bufs=1) as consts,
        tc.tile_pool(name="ct", bufs=6) as ct_pool,
        tc.tile_pool(name="ctxT", bufs=6) as ctxT_pool,
        tc.tile_pool(name="qn", bufs=5) as qn_pool,
        tc.tile_pool(name="sim", bufs=4) as sim_pool,
        tc.tile_pool(name="smalls", bufs=8) as small_pool,
        tc.tile_pool(name="gout", bufs=4) as g_pool,
        tc.tile_pool(name="ptp", bufs=2, space="PSUM") as ptp_pool,
        tc.tile_pool(name="psim", bufs=2, space="PSUM") as psim_pool,
        tc.tile_pool(name="pt4", bufs=1, space="PSUM") as pt4_pool,
        tc.tile_pool(name="pq2c", bufs=1, space="PSUM") as pq2c_pool,
        tc.tile_pool(name="pc2q", bufs=2, space="PSUM") as pc2q_pool,
    ):
        ident = consts.tile([128, 128], BF16)
        make_identity(nc, ident)
        # w packed host-side as [128, 6]: cols (w_c j0, w_c j1, w_q j0, w_q j1, w_m j0, w_m j1)
        wt = consts.tile([128, 6], BF16)
        nc.sync.dma_start(out=wt, in_=w_d[:])
        # question for ALL batches: [64, BL, 260], one DMA
        qn_all = consts.tile([Q, BL, DP], BF16)
        nc.sync.dma_start(out=qn_all, in_=qn_d[:])
        # w_q broadcast along c: rhs for folding the qq term into M1
        wqones = consts.tile([128, NDC, C], BF16)
        nc.vector.tensor_copy(wqones, wt[:, 2:4, None].broadcast_to([128, NDC, C]))

        st = {}  # per-batch live tiles, keyed by pipeline slot index

        def stage_load(s, b):
            """Input DMA issue — first in each iteration so the SP FIFO never
            queues it behind an output DMA still waiting on g assembly."""
            ct = ct_pool.tile([128, NCC, DP], BF16, tag="ct")
            nc.sync.dma_start(out=ct, in_=ctx_d[b])
            st[s] = dict(b=b, ct=ct)

        def stage_a(s, b):
            """Transposes: qnT, qnm, qnTw, ctxT."""
            qn_b = qn_all[:, b, :]
            ct = st[s]["ct"]

            ptq = ptp_pool.tile([128, 256], BF16, tag="ptp")
            for j in range(NDC):
                nc.tensor.transpose(
                    ptq[:, Q * j : Q * (j + 1)],
                    qn_b[:, 128 * j : 128 * (j + 1)],
                    ident[:Q, :Q],
                )
            qnT = qn_pool.tile([128, NDC, Q], BF16, tag="qnT")
            nc.vector.tensor_copy(
                qnT, ptq[:, : NDC * Q].rearrange("p (j q) -> p j q", q=Q)
            )

            # q-masked question (c2q rhs): rows scaled by qmask01; col 256
            # becomes qmask01 itself, so the c2q denominator is the masked sum
            qnm = qn_pool.tile([Q, D + 1], BF16, tag="qnm")
            nc.vector.tensor_mul(
                qnm, qn_b[:, : D + 1],
                qn_b[:, DP - 3 : DP - 2].broadcast_to([Q, D + 1]),
            )

            # qnTw [128, 2, 65]: cols 0..63 = qnT*w_m, col 64 = w_c (cq fold)
            qnTw = qn_pool.tile([128, NDC, M1R], BF16, tag="qnTw")
            nc.vector.tensor_mul(
                qnTw[:, :, :Q], qnT, wt[:, 4:6, None].broadcast_to([128, NDC, Q])
            )
            nc.gpsimd.tensor_copy(qnTw[:, :, Q], wt[:, 0:2])

            ctxT = []
            for j in range(NDC):
                cT = ctxT_pool.tile([128, C], BF16, tag=f"ctxT{j}")
                pt = ptp_pool.tile([128, C], BF16, tag="ptp")
                for i in range(NCC):
                    nc.tensor.transpose(
                        pt[:, 128 * i : 128 * (i + 1)],
                        ct[:, i, 128 * j : 128 * (j + 1)],
                        ident,
                    )
                nc.vector.tensor_copy(cT, pt)
                ctxT.append(cT)
            st[s] = dict(b=b, ct=ct, qnT=qnT, qnTw=qnTw, ctxT=ctxT, qnm=qnm)

        def stage_m1(s):
            """M1: simT [65, 512]; row 64 = cq; qq term folded in via w_q ⊗ 1.
            First/last matmuls cover the full [65, C] region; the qq
            sub-region matmuls accumulate in between."""
            d = st[s]
            qnT, qnTw, ctxT = d["qnT"], d["qnTw"], d["ctxT"]
            psim = psim_pool.tile([M1R, C], F32, tag="psim")
            nc.tensor.matmul(psim, qnTw[:, 0, :], ctxT[0], start=True, stop=False)
            for j in range(NDC):
                nc.tensor.matmul(
                    psim[:Q, :], qnT[:, j, :], wqones[:, j, :],
                    start=False, stop=False, skip_group_check=True,
                )
            nc.tensor.matmul(psim, qnTw[:, 1, :], ctxT[1], start=False, stop=True)
            d["psim"] = psim

        def stage_exp(s):
            """exp over everything (no max-subtraction needed: |sim| small).
            Masks applied after exp as 0/1 multiplies."""
            d = st[s]
            expT_raw = sim_pool.tile([M1R, C], BF16, tag="expT")
            nc.scalar.activation(expT_raw, d.pop("psim"), EXP, scale=1.0)
            d["expT_raw"] = expT_raw

        def stage_soft(s):
            """Re-transpose + e_col. (The q-mask lives in qnm, not in exp.)"""
            d = st[s]
            ct, expT_raw = d["ct"], d["expT_raw"]
            # re-transpose; e_col = max_q(exp) * exp(cq) * cmask01
            # (chunk stride padded to 66 so each PSUM write is 4B-aligned)
            pt4 = pt4_pool.tile([128, NCC, M1R + 1], BF16, tag="pt4")
            for i in range(NCC):
                nc.tensor.transpose(
                    pt4[:, i, :M1R], expT_raw[:, 128 * i : 128 * (i + 1)],
                    ident[:M1R, :M1R],
                )
            m_col = small_pool.tile([128, NCC], BF16, tag="mcol")
            nc.vector.reduce_max(m_col, pt4[:, :, :Q], axis=AX)
            e1 = small_pool.tile([128, NCC], BF16, tag="e1")
            nc.vector.tensor_mul(e1, m_col, pt4[:, :, Q])
            e_col = small_pool.tile([128, NCC], BF16, tag="ecol")
            nc.gpsimd.tensor_mul(e_col, e1, ct[:, :, DP - 3])
            d["e_col"] = e_col

        def stage_c(s):
            """q2c + c2q + g assembly + store."""
            d = st.pop(s)
            ct, expT_raw, e_col, qnm = d["ct"], d["expT_raw"], d["e_col"], d["qnm"]
            g_all = g_pool.tile([128, NCC, 3 * D], BF16, tag="gall")
            for i in range(NCC):
                pc2q = pc2q_pool.tile([128, D + 1], F32, tag="pc2q")
                nc.tensor.matmul(
                    pc2q, expT_raw[:Q, 128 * i : 128 * (i + 1)], qnm,
                    start=True, stop=True,
                )
                r_col = small_pool.tile([128, 1], F32, tag="rcol")
                nc.vector.reciprocal(r_col, pc2q[:, D : D + 1])
                # g2 = c2q (normalized) — ACT scaled copy from PSUM
                nc.scalar.activation(g_all[:, i, 0:D], pc2q[:, :D], COPY, scale=r_col)
                # g3 = ctx * c2q (Pool)
                nc.gpsimd.tensor_mul(
                    g_all[:, i, D : 2 * D], ct[:, i, :D], g_all[:, i, 0:D]
                )

            pq2c = pq2c_pool.tile([128, D + 1], F32, tag="pq2c")
            for i in range(NCC):
                # stride-0 lhsT: e_col chunk broadcast to all 128 columns, so
                # the PSUM rows come out already replicated across partitions
                nc.tensor.matmul(
                    pq2c, e_col[:, i : i + 1].broadcast_to([128, 128]),
                    ct[:, i, : D + 1],
                    start=(i == 0), stop=(i == NCC - 1),
                )
            rq = small_pool.tile([128, 1], F32, tag="rq")
            nc.vector.reciprocal(rq, pq2c[:, D : D + 1])
            gq = sim_pool.tile([128, D], BF16, tag="gq")  # q2c, normalized
            nc.scalar.activation(gq, pq2c[:, :D], COPY, scale=rq)
            for i in range(NCC):
                # g4 = ctx * q2c (DVE)
                nc.vector.tensor_mul(g_all[:, i, 2 * D : 3 * D], ct[:, i, :D], gq)
                if i % 2 == 1:  # store each completed half right away
                    nc.sync.dma_start(
                        out=g_d[d["b"], :, i - 1 : i + 1, :],
                        in_=g_all[:, i - 1 : i + 1, :],
                    )

        # Software pipeline. Every stage consumes only tiles produced a full
        # loop iteration earlier, so no engine stalls mid-stream on another
        # engine's in-flight work. Deepest stage emitted first (highest
        # scheduler priority).
        import os
        if os.environ.get("PIPE_DEPTH", "4") == "3":
            # depth 3: soft+C merged; c2q (independent of the e_col chain)
            # fills the PE gap while m_col/e1/e_col resolve
            def stage_softc(s):
                stage_soft(s)
                stage_c(s)

            def stage_am1(s):
                stage_a(s, s % BL)
                stage_m1(s)

            stages = [stage_softc, stage_exp, stage_am1]
        elif os.environ.get("PIPE_DEPTH", "4") == "5":
            stages = [
                stage_c,
                stage_soft,
                stage_exp,
                stage_m1,
                lambda s: stage_a(s, s % BL),
            ]
        else:
            # depth 4: M1 at the tail of stage A — its consumer (exp) only
            # reads psim in the next iteration, so a late M1 is harmless
            def stage_am1(s):
                stage_a(s, s % BL)
                stage_m1(s)

            stages = [stage_c, stage_soft, stage_exp, stage_am1]
        n = reps * BL
        depth = len(stages)
        for t in range(n + depth - 1):
            for k, stage in enumerate(stages):
                slot = t - (depth - 1 - k)
                if 0 <= slot < n:
                    stage(slot)


def build_module(compile=True, reps=1):
    nc = bacc.Bacc(trn_type="TRN2")
    ctx_d = nc.dram_tensor("context", [BL, 128, NCC, DP], BF16, kind="ExternalInput")
    qn_d = nc.dram_tensor("question", [Q, BL, DP], BF16, kind="ExternalInput")
    w_d = nc.dram_tensor("w", [128, 6], BF16, kind="ExternalInput")
    g_d = nc.dram_tensor("g", [BL, 128, NCC, 3 * D], BF16, kind="ExternalOutput")
    with tile.TileContext(nc) as tc:
        _emit(tc, ctx_d, qn_d, w_d, g_d, reps=reps)
    if compile:
        nc.compile()
    return nc


_NC_CACHE = None


def _get_module():
    global _NC_CACHE
    if _NC_CACHE is None:
        _NC_CACHE = build_module()
    return _NC_CACHE


def _pad_rows(x, mask01):
    """[N, S, 256] + [N, S] 0/1 mask -> [N, S, 260] with ones/mask columns."""
    n, s, d = x.shape
    out = np.zeros((n, s, DP), dtype=np.float32)
    out[:, :, :d] = x
    out[:, :, d] = 1.0
    out[:, :, d + 1] = mask01
    return out


def make_in_maps(context, question, context_mask, question_mask, w):
    context = np.asarray(context, dtype=np.float32)
    question = np.asarray(question, dtype=np.float32)
    cm = (np.asarray(context_mask) != 0).astype(np.float32)
    qm = (np.asarray(question_mask) != 0).astype(np.float32)
    # ctx: [B, C, 260] -> [B, 128, 4, 260] so DMA lines are contiguous
    ctx_p = _pad_rows(context, cm).reshape(B, NCC, 128, DP).transpose(0, 2, 1, 3)
    ctx_p = np.ascontiguousarray(ctx_p).astype(NPBF16)
    # qn: [B, Q, 260] -> per core [Q, BL, 260]
    qn_p = _pad_rows(question, qm).astype(NPBF16)
    # [768] -> [128, 6] columns (w_c j0, w_c j1, w_q j0, w_q j1, w_m j0, w_m j1)
    w_p = np.ascontiguousarray(
        np.asarray(w, dtype=np.float32).reshape(3, NDC, 128).transpose(2, 0, 1).reshape(128, 6)
    ).astype(NPBF16)
    in_maps = []
    for k in range(N_CORES):
        sl = slice(k * BL, (k + 1) * BL)
        in_maps.append(
            {
                "context": np.ascontiguousarray(ctx_p[sl]),
                "question": np.ascontiguousarray(qn_p[sl].transpose(1, 0, 2)),
                "w": w_p,
            }
        )
    return in_maps


def kernel(context, question, context_mask, question_mask, w):
    nc = _get_module()
    in_maps = make_in_maps(context, question, context_mask, question_mask, w)
    res = run_bass_kernel_spmd(nc, in_maps, list(range(N_CORES)))
    out = np.empty((B, C, 4 * D), dtype=np.float32)
    # g1 = ctx block: verbatim input copy, assembled during unshard
    out[:, :, :D] = np.asarray(context, dtype=np.float32)
    for k in range(N_CORES):
        gk = np.asarray(res.results[k]["g"]).astype(np.float32)  # [BL,128,4,768]
        out[k * BL : (k + 1) * BL, :, D:] = (
            gk.transpose(0, 2, 1, 3).reshape(BL, C, 3 * D)
        )
    return out
